# revision 4
# baseline (speedup 1.0000x reference)
"""Trainium2 Bass kernel for the dense MoE layer (nn_MoELayer_74371653698164).

Reference computation (fp32):
    gate  = softmax(x @ Wg + bg)                    # [N, E]
    out   = sum_e gate[:, e] * (x @ We[e] + be[e])  # [N, D_OUT]

Strategy:
  - Data-parallel over tokens: each of the 8 cores gets N/8 = 1024 tokens and
    the full expert/gate weights. No collectives.
  - x is pre-transposed on the host (a weights-style layout change), so the
    contraction dim lands on partitions with a single DMA and no on-device
    transpose pass.
  - Softmax is factored: out = r * (sum_e exp_e * (x @ We[e] + be[e])) with
    r = 1 / sum_e exp_e. Logits are computed TRANSPOSED ([E, tok], E on
    partitions) so the bias add is a per-partition scalar op and exp is one
    activation over all tokens; logits here are ~N(0,1) so max-subtraction
    is unnecessary in fp32.
  - Per expert: stream We[e] from HBM, run 128x512 matmuls into a 2-bank
    [128,1024] PSUM pair, and fold the gate in with one DVE FMA per
    (expert, token-tile): acc = psum * exp[:, e] + acc. The bias term
    (exp @ be, one K=8 matmul pair per token tile) initializes acc.
  - Matmul operands are float32r: full fp32 data that the PE streams at
    1 cycle/row for N>=256 (vs 4 for strict fp32), near-fp32 precision.
  - Instruction count is minimized throughout (batched drains, paired PSUM
    banks, single big DMAs) — both dispatch overhead and real-HW sync cost
    scale with it.

kernel(**inputs) takes the FULL unsharded inputs and returns the FULL output.
"""
import os
from contextlib import ExitStack

import numpy as np

import bass_rust
import concourse.bass as bass
import concourse.mybir as mybir
import concourse.tile as tile
from concourse.bass_utils import run_bass_kernel_spmd
from concourse.masks import make_identity
from concourse.vector_clock import ScopedClock

# Problem shape (hardcoded per harness contract).
N_TOKENS, D_IN, D_OUT, E = 8192, 1024, 1024, 8
NCORES = 8
TOK = N_TOKENS // NCORES  # tokens per core
P = 128                   # partitions
KT = D_IN // P            # contraction tiles
TT = TOK // P             # token tiles per core
FH = 512                  # max fp32 matmul free dim (one PSUM bank)

# "f32r" (default): fp32 data, PE in float32r mode (fast, ~fp32 precision)
# "f32": strict fp32 matmuls (4x slower PE)
# "bf16": bf16 inputs (half DMA traffic, ~3e-3 rel err)
MODE = os.environ.get("MOE_KERNEL_MODE", "f32r")

_F32 = mybir.dt.float32
_F32R = mybir.dt.float32r
_BF16 = mybir.dt.bfloat16


class _ChunkedDrainTileContext(tile.TileContext):
    """TileContext adapted to a walrus that allows ONE sync wait per
    instruction ("Too many sync wait commands", CoreV3GenImpl setupSyncWait).

    Stock Tile attaches up to ~3 waits to an instruction (and the whole
    global-clock wait set to the tail drain). Every extra wait is hoisted
    onto a same-engine InstNoOp carrier emitted immediately before the
    instruction, so the engine's sequencer observes the sems in order.
    """

    _HOIST_WAITS = os.environ.get("MOE_HOIST_WAITS", "0") == "1"

    def __init__(self, *a, **kw):
        super().__init__(*a, **kw)
        self._last_by_engine = {}

    def _add_instruction(self, inst):
        si = getattr(inst, "sync_info", None)
        if si is not None and si.on_wait and len(si.on_wait) > 1:
            waits = list(si.on_wait)
            # Optionally park one extra wait on the immediately preceding
            # same-engine instruction when it carries no waits/updates of its
            # own: the wait just fires one slot earlier in the same stream.
            if self._HOIST_WAITS and len(waits) == 2:
                prev = self._last_by_engine.get(inst.engine)
                psi = getattr(prev, "sync_info", None) if prev is not None else None
                if prev is not None and (
                    psi is None or (not psi.on_wait and not psi.on_update)
                ):
                    prev.sync_info = bass_rust.SyncInfo(
                        on_wait=[waits[0]], on_update=[])
                    waits = waits[1:]
            for w in waits[:-1]:
                nop = mybir.InstNoOp(
                    name=self.nc.get_next_instruction_name(), ins=[], outs=[]
                )
                nop.engine = inst.engine
                nop.bass_nofuse = True
                nop.sync_info = bass_rust.SyncInfo(on_wait=[w], on_update=[])
                super()._add_instruction(nop)
            inst.sync_info = bass_rust.SyncInfo(
                on_wait=[waits[-1]], on_update=list(si.on_update or [])
            )
        self._last_by_engine[inst.engine] = inst
        super()._add_instruction(inst)

    def _drain_and_barrier(self, tick_clock, wait_clock):
        drain_inst = self.nc.sync.drain()
        wait_clock.add_sem_waits(
            drain_inst.ins, ScopedClock({None: tick_clock.global_clock})
        )
        si = drain_inst.ins.sync_info
        waits = list(si.on_wait or []) if si is not None else []
        if len(waits) > 1:
            drain_inst.ins.sync_info = bass_rust.SyncInfo(
                on_wait=waits[:1], on_update=list(si.on_update or [])
            )
            for w in waits[1:]:
                extra = self.nc.sync.drain()
                extra.ins.sync_info = bass_rust.SyncInfo(on_wait=[w], on_update=[])

        self.nc.all_engine_barrier()
        assert self.sems is not None
        popped = self.nc._tile_sem_poison_stack.pop()
        assert popped is self._sem_poison
        self.nc.clear_and_free_semaphores(list(self.sems.allocated().values()))
        self.nc.all_engine_barrier()


def build_nc(mode: str = MODE, reps: int = 1, internal_io: bool = False,
             n_experts: int = E, do_gate: bool = True, do_bias: bool = True,
             do_fma: bool = True, do_store: bool = True, do_mm: bool = True,
             we_loads: int | None = None, fh: int = FH) -> bass.Bass:
    """Build the per-core Bass program.

    reps: repeat the compute body (timing harnesses amortize dispatch
    overhead); internal_io: inputs live in internal DRAM seeded on-device
    (timing without host transfers); remaining flags ablate stages.
    """
    mmdt = {"bf16": _BF16, "f32": _F32, "f32r": _F32R}[mode]

    nc = bass.Bass()
    kind_in = {} if internal_io else {"kind": "ExternalInput"}
    xT_d = nc.dram_tensor("xT", [D_IN, TOK], mmdt, **kind_in)
    We_d = nc.dram_tensor("We", [E, D_IN, D_OUT], mmdt, **kind_in)
    be_d = nc.dram_tensor("be", [E, D_OUT], mmdt, **kind_in)
    Wg_d = nc.dram_tensor("Wg", [D_IN, E], mmdt, **kind_in)
    bg_d = nc.dram_tensor("bg", [E], _F32, **kind_in)
    if internal_io:
        out_d = nc.dram_tensor("out", [TOK, D_OUT], _F32)
        probe_d = nc.dram_tensor("probe", [P, P], _F32, kind="ExternalOutput")
    else:
        out_d = nc.dram_tensor("out", [TOK, D_OUT], _F32, kind="ExternalOutput")
        probe_d = None

    with _ChunkedDrainTileContext(nc) as tc, ExitStack() as ctx:
        singles = ctx.enter_context(tc.tile_pool(name="singles", bufs=1))
        wepool = ctx.enter_context(tc.tile_pool(name="we", bufs=2))
        # PSUM budget: pair pool 2x[128,1024] = 4 banks; ps_b holds the gate
        # logits tile (2 banks) + the exp-transpose staging tile (1 bank).
        ps_pair = ctx.enter_context(tc.tile_pool(name="ps_pair", bufs=2,
                                                 space="PSUM"))
        ps_b = ctx.enter_context(tc.tile_pool(name="ps_b", bufs=1,
                                              space="PSUM"))

        if internal_io:
            # Seed internal inputs on-device with benign constants (values
            # are irrelevant to timing; softmax logits stay small/finite).
            seed = singles.tile([P, D_OUT], _F32, tag="seed")
            nc.vector.memset(seed[:], 0.005)
            if mmdt != _F32:
                seed_mm = singles.tile([P, D_OUT], mmdt, tag="seedmm")
                nc.vector.memset(seed_mm[:], 0.005)
            else:
                seed_mm = seed

            def rep_src(n_rep):
                s = seed_mm[:, :].opt()
                return bass.AP(tensor=s.tensor, offset=s.offset,
                               ap=[[s.ap[0][0], P], [0, n_rep], [1, D_OUT]])

            nc.sync.dma_start(xT_d.rearrange("(k p) n -> p k n", p=P),
                              rep_src(KT))
            for e in range(E):
                nc.sync.dma_start(We_d[e].rearrange("(k p) o -> p k o", p=P),
                                  rep_src(KT))
            nc.sync.dma_start(be_d[:, :], seed_mm[0:E, :])
            nc.sync.dma_start(Wg_d.rearrange("(k p) e -> p k e", p=P),
                              seed_mm[:, 0:KT * E].rearrange(
                                  "p (k e) -> p k e", k=KT))
            nc.sync.dma_start(bg_d[:], seed[0, 0:E])

        # small identity for the [E,tok] -> [tok,E] exp transpose
        ident8f = singles.tile([E, E], _F32, tag="id8f")
        make_identity(nc, ident8f)
        if mmdt != _F32:
            ident8 = singles.tile([E, E], mmdt, tag="id8")
            nc.scalar.copy(ident8[:], ident8f[:])
        else:
            ident8 = ident8f

        wg_sb = singles.tile([P, KT, E], mmdt, tag="wg")
        nc.sync.dma_start(wg_sb[:], Wg_d.rearrange("(k p) e -> p k e", p=P))
        bg_col = singles.tile([E, 1], _F32, tag="bg")
        nc.sync.dma_start(bg_col[:], bg_d[:])
        be_sb = singles.tile([E, D_OUT], mmdt, tag="be")
        nc.sync.dma_start(be_sb[:], be_d[:, :])

        acc = None
        exp_tok = None
        expT = None
        r_tok = None
        for _ in range(reps):
            xT = singles.tile([P, KT, TOK], mmdt, tag="xT")
            nc.sync.dma_start(xT[:], xT_d.rearrange("(k p) n -> p k n", p=P))
            acc = singles.tile([P, TT, D_OUT], _F32, tag="acc")

            if do_gate:
                # logits^T [E, tok] in PSUM (E on partitions), bias add as a
                # per-partition scalar, exp over all tokens at once.
                pg = ps_b.tile([E, TOK], _F32, tag="g")
                for k in range(KT):
                    for h in range(TOK // FH):
                        nc.tensor.matmul(
                            pg[:, h * FH:(h + 1) * FH], wg_sb[:, k, :],
                            xT[:, k, h * FH:(h + 1) * FH],
                            start=(k == 0), stop=(k == KT - 1),
                        )
                ltT = singles.tile([E, TOK], _F32, tag="ltT")
                nc.vector.tensor_scalar_add(ltT[:], pg[:], bg_col[:])
                expT = singles.tile([E, TOK], mmdt, tag="expT")
                nc.scalar.activation(expT[:], ltT[:],
                                     mybir.ActivationFunctionType.Exp)
                # transpose exp to token layout: 8 blocks into one PSUM bank
                ptr = ps_b.tile([P, TT * E], mmdt, tag="tr")
                for i in range(TT):
                    nc.tensor.transpose(ptr[:, i * E:(i + 1) * E],
                                        expT[:, i * P:(i + 1) * P], ident8[:])
                exp_tok = singles.tile([P, TT, E], _F32, tag="exptok")
                nc.scalar.copy(exp_tok.rearrange("p a b -> p (a b)"), ptr[:])
                s_tok = singles.tile([P, TT, 1], _F32, tag="stok")
                nc.vector.reduce_sum(s_tok[:], exp_tok[:],
                                     axis=mybir.AxisListType.X)
                r_tok = singles.tile([P, TT, 1], _F32, tag="rtok")
                nc.vector.reciprocal(r_tok[:], s_tok[:])

            if do_bias:
                # acc init: (unnormalized) exp @ be
                for i in range(TT):
                    pb = ps_pair.tile([P, D_OUT], _F32, tag="pair")
                    for h in range(D_OUT // FH):
                        nc.tensor.matmul(
                            pb[:, h * FH:(h + 1) * FH],
                            expT[:, i * P:(i + 1) * P],
                            be_sb[:, h * FH:(h + 1) * FH],
                            start=True, stop=True,
                        )
                    nc.scalar.copy(acc[:, i, :], pb[:])

            # experts: acc += exp[:, e] * (x @ We[e])
            we = None
            for e in range(n_experts):
                if we_loads is None or e < we_loads:
                    we = wepool.tile([P, KT, D_OUT], mmdt, tag="we")
                    nc.sync.dma_start(
                        we[:], We_d[e].rearrange("(k p) o -> p k o", p=P))
                for i in range(TT if do_mm else 0):
                    isl = slice(i * P, (i + 1) * P)
                    pm = ps_pair.tile([P, D_OUT], _F32, tag="pair")
                    for k in range(KT):
                        for h in range(D_OUT // fh):
                            nc.tensor.matmul(
                                pm[:, h * fh:(h + 1) * fh], xT[:, k, isl],
                                we[:, k, h * fh:(h + 1) * fh],
                                start=(k == 0), stop=(k == KT - 1),
                            )
                    if do_fma:
                        nc.vector.scalar_tensor_tensor(
                            out=acc[:, i, :], in0=pm[:],
                            scalar=exp_tok[:, i, e:e + 1], in1=acc[:, i, :],
                            op0=mybir.AluOpType.mult, op1=mybir.AluOpType.add,
                        )

            if do_gate and do_fma:
                # normalize: acc *= 1/sum(exp), broadcast over D_OUT
                r = r_tok[:, :, 0:1].opt()
                rb = bass.AP(tensor=r.tensor, offset=r.offset,
                             ap=[r.ap[0], r.ap[1], [0, D_OUT]])
                nc.vector.tensor_mul(acc[:], acc[:], rb)

            if do_store:
                nc.sync.dma_start(out_d.rearrange("(i p) o -> p i o", p=P),
                                  acc[:])

        if internal_io:
            nc.sync.dma_start(probe_d[:, :], acc[:, 0, 0:P])

    return nc


_NC_CACHE: dict = {}


def _get_nc(mode: str, reps: int = 1) -> bass.Bass:
    key = (mode, reps)
    if key not in _NC_CACHE:
        _NC_CACHE[key] = build_nc(mode, reps)
    return _NC_CACHE[key]


def make_in_maps(x, We, be, Wg, bg, mode: str = MODE):
    import ml_dtypes

    dt_np = ml_dtypes.bfloat16 if mode == "bf16" else np.float32
    We_c = np.ascontiguousarray(We, dtype=dt_np)
    be_c = np.ascontiguousarray(be, dtype=dt_np)
    Wg_c = np.ascontiguousarray(Wg, dtype=dt_np)
    bg_c = np.ascontiguousarray(bg, dtype=np.float32)
    in_maps = []
    for c in range(NCORES):
        xs = np.asarray(x[c * TOK:(c + 1) * TOK], dtype=dt_np)
        in_maps.append({
            "xT": np.ascontiguousarray(xs.T),
            "We": We_c,
            "be": be_c,
            "Wg": Wg_c,
            "bg": bg_c,
        })
    return in_maps


def kernel(x, We, be, Wg, bg):
    nc = _get_nc(MODE)
    in_maps = make_in_maps(x, We, be, Wg, bg, MODE)
    res = run_bass_kernel_spmd(nc, in_maps, list(range(NCORES)))
    out = np.concatenate([res.results[c]["out"] for c in range(NCORES)], axis=0)
    return out.astype(np.float32)



# revision 6
# speedup vs baseline: 2.0035x; 2.0035x over previous
"""Raw-Bass bf16 MoE kernel (v3) — minimal instruction count.

This environment executes ~1 instruction per ~35-70us regardless of content
(measured; see micro.py), so the kernel is designed to minimize the number of
EXECUTED instructions:
  - bf16 matmuls (measured ~25-40% cheaper than f32r; rel err ~4e-3 << 2e-2)
  - no Tile framework: semaphore waits/updates are attached directly to the
    instructions that need them (zero extra sync instructions, except one
    NoOp carrier per expert for the double-wait case)
  - all aux work in the fewest, widest ops possible

Dataflow per core (data-parallel over tokens, TOK=1024 per core):
  gate:    logitsT[E,TOK] = Wg^T-stationary matmuls; +bg; exp (ACT)
           transpose exp -> token layout; sum_E; recip -> r_tok
           gtok = exp_tok * r_tok  (normalized gate, [128,TT,E])
  bias:    pb[i] = exp @ be  (PE);  acc[i] = pb[i] * r_tok[i]  (DVE, normalized)
  experts: per (e,i): 16 bf16 matmuls -> pm pair; TSP: acc[i] += pm * gtok[i,e]
  store:   one DMA of acc.
"""
from contextlib import ExitStack

import numpy as np

import concourse.bass as bass
import concourse.mybir as mybir

N_TOKENS, D_IN, D_OUT, E = 8192, 1024, 1024, 8
NCORES = 8
TOK = N_TOKENS // NCORES
P = 128
KT = D_IN // P    # 8 contraction tiles
TT = TOK // P     # 8 token tiles
FH = 512

_F32 = mybir.dt.float32
_BF16 = mybir.dt.bfloat16


def build_v3(reps: int = 1, internal_io: bool = False) -> bass.Bass:
    nc = bass.Bass()
    kind_in = {} if internal_io else {"kind": "ExternalInput"}
    xT_d = nc.dram_tensor("xT", [D_IN, TOK], _BF16, **kind_in)
    We_d = nc.dram_tensor("We", [E, D_IN, D_OUT], _BF16, **kind_in)
    be_d = nc.dram_tensor("be", [E, D_OUT], _BF16, **kind_in)
    Wg_d = nc.dram_tensor("Wg", [D_IN, E], _BF16, **kind_in)
    bg_d = nc.dram_tensor("bg", [E], _F32, **kind_in)
    id_d = nc.dram_tensor("ident", [E, E], _BF16, **kind_in)
    if internal_io:
        out_d = nc.dram_tensor("out", [TOK, D_OUT], _F32)
        probe_d = nc.dram_tensor("probe", [P, P], _F32, kind="ExternalOutput")
    else:
        out_d = nc.dram_tensor("out", [TOK, D_OUT], _F32, kind="ExternalOutput")
        probe_d = None

    ctx = ExitStack()
    # SBUF ([partition, ...]; bf16 unless noted)
    xT = ctx.enter_context(nc.sbuf_tensor("xTs", [P, KT, TOK], _BF16))
    we = ctx.enter_context(nc.sbuf_tensor("wes", [P, 2, KT, D_OUT], _BF16))
    acc = ctx.enter_context(nc.sbuf_tensor("accs", [P, TT, D_OUT], _F32))
    wg = ctx.enter_context(nc.sbuf_tensor("wgs", [P, KT, E], _BF16))
    bgc = ctx.enter_context(nc.sbuf_tensor("bgc", [E, 1], _F32))
    bes = ctx.enter_context(nc.sbuf_tensor("bes", [E, D_OUT], _BF16))
    ident = ctx.enter_context(nc.sbuf_tensor("idents", [E, E], _BF16))
    ltT = ctx.enter_context(nc.sbuf_tensor("ltT", [E, TOK], _F32))
    expT = ctx.enter_context(nc.sbuf_tensor("expT", [E, TOK], _BF16))
    exptok = ctx.enter_context(nc.sbuf_tensor("exptok", [P, TT, E], _F32))
    stok = ctx.enter_context(nc.sbuf_tensor("stok", [P, TT, 1], _F32))
    rtok = ctx.enter_context(nc.sbuf_tensor("rtok", [P, TT, 1], _F32))
    gtok = ctx.enter_context(nc.sbuf_tensor("gtok", [P, TT, E], _F32))
    if internal_io:
        seedf = ctx.enter_context(nc.sbuf_tensor("seedf", [P, D_OUT], _F32))
        seedb = ctx.enter_context(nc.sbuf_tensor("seedb", [P, D_OUT], _BF16))
    # PSUM: 4 pairs of banks as one tensor [128, 4, 1024] f32 (all 8 banks)
    pm = ctx.enter_context(nc.psum_tensor("pm", [P, 4, 1024], _F32))
    # gate logits view [E, 1024] on pair 0; transpose staging on pair 1
    pg = pm[0:E, 0, :]
    ptr = pm[:, 1, 0:32].bitcast(_BF16)  # [128, 64] bf16 in bank 2

    # Semaphores. DMA completions are UNORDERED across in-flight DMAs, so
    # each dependency group gets its own semaphore; a waiter's threshold is
    # only ever satisfied by the exact DMAs it needs.
    semSU = nc.alloc_semaphore("semSU")    # setup + seed DMAs
    semX = nc.alloc_semaphore("semX")      # xT loads (1/rep)
    semW = [nc.alloc_semaphore("semW0"), nc.alloc_semaphore("semW1")]
    semPE = nc.alloc_semaphore("semPE")    # expert-chain completions
    semPEg = nc.alloc_semaphore("semPEg")  # gate/tr/bias PE milestones
    semDVE = nc.alloc_semaphore("semDVE")  # DVE op completions

    su = 0     # semSU cumulative
    pe = 0     # semPE cumulative (expert chain ends)
    peg = 0    # semPEg cumulative
    dve = 0    # semDVE cumulative

    def dma(dst, src, sem, val, wait=None):
        inst = nc.sync.dma_start(dst, src)
        if wait is not None:
            inst.wait_op(wait[0], wait[1], "sem-ge")
        inst.then_inc(sem, 16)
        return val + 16

    def dma_su(dst, src, wait=None):
        nonlocal su
        su = dma(dst, src, semSU, su, wait=wait)
        return su

    if internal_io:
        nc.vector.memset(seedf[:, :], 0.005)
        nc.vector.memset(seedb[:, :], 0.005)
        nc.vector.memset(seedf[:, :], 0.005).then_inc(semDVE, 1)
        dve += 1

        def rep_src(n_rep):
            s = seedb[:, :].opt()
            return bass.AP(tensor=s.tensor, offset=s.offset,
                           ap=[[s.ap[0][0], P], [0, n_rep], [1, D_OUT]])

        # seeds wait on the memsets via semDVE; later SP DMAs dispatch
        # in sequencer order, so only the first needs the wait
        dma_su(xT_d.rearrange("(k p) n -> p k n", p=P), rep_src(KT),
               wait=(semDVE, dve))
        for e in range(E):
            dma_su(We_d[e].rearrange("(k p) o -> p k o", p=P), rep_src(KT))
        dma_su(be_d[:, :], seedb[0:E, :])
        dma_su(Wg_d.rearrange("(k p) e -> p k e", p=P),
               seedb[:, 0:KT * E].rearrange("p (k e) -> p k e", k=KT))
        dma_su(bg_d[:], seedf[0, 0:E])
        dma_su(id_d[:, :], seedb[0:E, 0:E])

    # ---- setup loads (once) ----
    dma_su(wg[:, :, :], Wg_d.rearrange("(k p) e -> p k e", p=P))
    dma_su(bgc[:, :], bg_d[:])
    dma_su(bes[:, :], be_d[:, :])
    dma_su(ident[:, :], id_d[:, :])
    setup_su = su

    last_tsp_dve = None   # semDVE value of final TSP of previous rep
    xv = 0                # semX cumulative
    wv = [0, 0]           # semW slot cumulative

    for r in range(reps):
        # xT load; WAR on xT + all psum banks proven free via last rep's
        # TSPs. In rep 0 the free wait slot instead covers setup/seeds.
        xt_wait = ((semDVE, last_tsp_dve) if last_tsp_dve is not None
                   else (semSU, setup_su))
        xv = dma(xT[:, :, :], xT_d.rearrange("(k p) n -> p k n", p=P),
                 semX, xv, wait=xt_wait)

        we_val = []
        for e in range(E):
            # WAR on we slot e%2: expert e-2's chains done (tile numbering is
            # global across reps: 8 tiles per expert, 64 per rep)
            need = 64 * r + 8 * (e - 2) + 8
            wait = (semPE, need) if need > 0 else (semSU, setup_su)
            wv[e % 2] = dma(we[:, e % 2, :, :],
                            We_d[e].rearrange("(k p) o -> p k o", p=P),
                            semW[e % 2], wv[e % 2], wait=wait)
            we_val.append(wv[e % 2])

        # ---- gate logits (PE): pg[E, TOK] = sum_k wg[k].T @ xT[k] ----
        for k in range(KT):
            for h in range(TOK // FH):
                inst = nc.tensor.matmul(
                    pg[:, h * FH:(h + 1) * FH], wg[:, k, :],
                    xT[:, k, h * FH:(h + 1) * FH],
                    start=(k == 0), stop=(k == KT - 1))
                if k == 0 and h == 0:
                    # xT (and transitively all setup DMAs) loaded
                    inst.wait_op(semX, xv, "sem-ge")
                if k == KT - 1 and h == TOK // FH - 1:
                    inst.then_inc(semPEg, 1)
        peg += 1
        gate_peg = peg

        # ---- DVE: ltT = pg + bg (per-partition scalar) ----
        inst = nc.vector.tensor_scalar_add(ltT[:, :], pg, bgc[:, :])
        inst.wait_op(semPEg, gate_peg, "sem-ge")
        inst.then_inc(semDVE, 1)
        dve += 1
        tsa_dve = dve

        # ---- ACT: expT = exp(ltT), bf16 out ----
        inst = nc.scalar.activation(expT[:, :], ltT[:, :],
                                    mybir.ActivationFunctionType.Exp)
        inst.wait_op(semDVE, tsa_dve, "sem-ge")
        inst.then_inc(semPEg, 1)  # reuse semPEg lane for ACT->PE handoff
        peg += 1
        exp_peg = peg

        # ---- PE: transpose expT into token-layout staging (bank 2) ----
        for i in range(TT):
            inst = nc.tensor.transpose(ptr[:, i * E:(i + 1) * E],
                                       expT[:, i * P:(i + 1) * P],
                                       ident[:, :])
            if i == 0:
                inst.wait_op(semPEg, exp_peg, "sem-ge")
            if i == TT - 1:
                inst.then_inc(semPEg, 1)
        peg += 1
        tr_peg = peg

        # ---- bias matmuls (PE): pb[i] = exp_block[i].T-stationary @ be ----
        # pb uses pairs 2,3 (banks 4-7), rotating per i; mm(i) must wait for
        # the DVE bias-init of i-2 before clobbering its pair. DVE incs this
        # rep: tsa (dve), copy/reduce/recip/mul (dve+1..4), bias-init(j)
        # (dve+5+j), so bias-init(i-2) completes at semDVE == dve + 3 + i.
        for i in range(TT):
            pb = pm[:, 2 + (i % 2), :]
            for h in range(2):
                inst = nc.tensor.matmul(pb[:, h * FH:(h + 1) * FH],
                                        expT[:, i * P:(i + 1) * P],
                                        bes[:, h * FH:(h + 1) * FH],
                                        start=True, stop=True)
                if h == 0 and i >= 2:
                    inst.wait_op(semDVE, dve + 3 + i, "sem-ge")
                if h == 1:
                    inst.then_inc(semPEg, 1)
            peg += 1

        # ---- DVE chain: exp_tok copy, sum, recip, gtok ----
        # Same-engine RAW also needs sem sync on this HW: each DVE op incs
        # semDVE and the next dependent one waits on that value. A wait on a
        # later semDVE value transitively covers all earlier DVE writes and
        # (because the store also incs semDVE) the previous rep's store of
        # acc.
        base = dve
        copy_v, reduce_v, recip_v, mul_v = base + 1, base + 2, base + 3, base + 4
        # write exptok through its canonical 3D AP (the race detector treats
        # reshaped write-views as separate shadow regions); reshape the
        # source instead.
        p3 = ptr[:, :].opt()
        ptr3 = bass.AP(tensor=p3.tensor, offset=p3.offset,
                       ap=[p3.ap[0], [E, TT], [1, E]])
        inst = nc.vector.tensor_copy(exptok[:, :, :], ptr3)
        inst.wait_op(semPEg, tr_peg, "sem-ge")
        inst.then_inc(semDVE, 1)
        inst = nc.vector.reduce_sum(stok[:, :, :], exptok[:, :, :],
                                    axis=mybir.AxisListType.X)
        inst.wait_op(semDVE, copy_v, "sem-ge")
        inst.then_inc(semDVE, 1)
        inst = nc.vector.reciprocal(rtok[:, :, :], stok[:, :, :])
        inst.wait_op(semDVE, reduce_v, "sem-ge")
        inst.then_inc(semDVE, 1)
        r_ap = rtok[:, :, 0:1].opt()
        rb = bass.AP(tensor=r_ap.tensor, offset=r_ap.offset,
                     ap=[r_ap.ap[0], r_ap.ap[1], [0, E]])
        inst = nc.vector.tensor_mul(gtok[:, :, :], exptok[:, :, :], rb)
        inst.wait_op(semDVE, recip_v, "sem-ge")
        inst.then_inc(semDVE, 1)
        dve = mul_v

        # ---- DVE: acc[i] = pb[i] * r_tok[i]  (normalized bias init) ----
        for i in range(TT):
            pb = pm[:, 2 + (i % 2), :]
            inst = nc.vector.tensor_scalar_mul(acc[:, i, :], pb,
                                               rtok[:, i, 0:1])
            inst.wait_op(semPEg, tr_peg + 1 + i, "sem-ge")
            inst.then_inc(semDVE, 1)
            dve += 1
        bias_init_done_dve = dve

        # ---- experts ----
        # tile t = e*TT + i (within rep); psum pair = t % 4; 4-deep pipeline
        tsp_dve_of_tile = {}
        for e in range(E):
            # we[e] loaded guard: one NoOp carrier per expert (PE)
            nc.tensor.nop().wait_op(semW[e % 2], we_val[e], "sem-ge")
            for i in range(TT):
                t = e * TT + i
                pair = pm[:, t % 4, :]
                isl = slice(i * P, (i + 1) * P)
                first_wait = None
                if t >= 4:
                    first_wait = (semDVE, tsp_dve_of_tile[t - 4])
                elif e == 0:
                    # pairs 0,1 freed by gate/tr consumers; pairs 2,3 by
                    # bias inits. Conservative single wait: all bias inits.
                    first_wait = (semDVE, bias_init_done_dve)
                for k in range(KT):
                    for h in range(2):
                        inst = nc.tensor.matmul(
                            pair[:, h * FH:(h + 1) * FH],
                            xT[:, k, isl],
                            we[:, e % 2, k, h * FH:(h + 1) * FH],
                            start=(k == 0), stop=(k == KT - 1))
                        if k == 0 and h == 0 and first_wait is not None:
                            inst.wait_op(first_wait[0], first_wait[1],
                                         "sem-ge")
                        if k == KT - 1 and h == 1:
                            inst.then_inc(semPE, 1)
                pe += 1
                # TSP on DVE: acc[i] += pm * gtok[i, e]
                inst = nc.vector.scalar_tensor_tensor(
                    out=acc[:, i, :], in0=pair, scalar=gtok[:, i, e:e + 1],
                    in1=acc[:, i, :],
                    op0=mybir.AluOpType.mult, op1=mybir.AluOpType.add)
                inst.wait_op(semPE, pe, "sem-ge")
                inst.then_inc(semDVE, 1)
                dve += 1
                tsp_dve_of_tile[t] = dve

        last_tsp_dve = dve

        # ---- store ----
        inst = nc.sync.dma_start(out_d.rearrange("(i p) o -> p i o", p=P),
                                 acc[:, :, :])
        inst.wait_op(semDVE, last_tsp_dve, "sem-ge")
        inst.then_inc(semDVE, 16)
        dve += 16

    if internal_io:
        inst = nc.sync.dma_start(probe_d[:, :], acc[:, 0, 0:P])
        inst.wait_op(semDVE, dve, "sem-ge")
        inst.then_inc(semDVE, 16)
        dve += 16
    # final quiesce so the NEFF doesn't retire before the stores complete
    nc.sync.wait_ge(semDVE, dve)

    ctx.close()
    return nc


def make_in_maps_v3(x, We, be, Wg, bg):
    import ml_dtypes

    bf = ml_dtypes.bfloat16
    We_c = np.ascontiguousarray(We, dtype=bf)
    be_c = np.ascontiguousarray(be, dtype=bf)
    Wg_c = np.ascontiguousarray(Wg, dtype=bf)
    bg_c = np.ascontiguousarray(bg, dtype=np.float32)
    id_c = np.eye(E, dtype=bf)
    in_maps = []
    for c in range(NCORES):
        xs = np.asarray(x[c * TOK:(c + 1) * TOK], dtype=bf)
        in_maps.append({
            "xT": np.ascontiguousarray(xs.T),
            "We": We_c,
            "be": be_c,
            "Wg": Wg_c,
            "bg": bg_c,
            "ident": id_c,
        })
    return in_maps


_NC_CACHE = {}


def kernel(x, We, be, Wg, bg):
    from concourse.bass_utils import run_bass_kernel_spmd

    if "v3" not in _NC_CACHE:
        _NC_CACHE["v3"] = build_v3()
    nc = _NC_CACHE["v3"]
    in_maps = make_in_maps_v3(x, We, be, Wg, bg)
    res = run_bass_kernel_spmd(nc, in_maps, list(range(NCORES)))
    out = np.concatenate([res.results[c]["out"] for c in range(NCORES)],
                         axis=0)
    return out.astype(np.float32)
